# revision 48
# baseline (speedup 1.0000x reference)
"""Trainium2 Bass kernel for MultiHead GQA attention (B=1, S=2048, D=1024,
16 q-heads / 4 kv-heads, GQA group 4, RoPE, causal).  bf16 compute, f32 PSUM.

Sharding: tensor-parallel over heads. Core c (of 8) computes 2 query heads
{g, g+4} (c even) or {g+8, g+12} (c odd) with g = c//2, which both attend kv
head g (jnp.tile GQA semantics: q-head h uses kv head h % 4). Wq/Wk/Wv are
column-sharded, Wo row-sharded; each core produces a partial [D, S] bf16
output (transposed) and the host reduces the 8 partials and adds bo.

v3 schedule:
  - input DMAs form a global chain (depth 3) so s-tiles arrive in need
    order instead of packet-round-robin arrival inversion
  - scores for jt+1 issue BEFORE the PV matmuls of jt, so the PE streams
    through the exp latency instead of waiting it out every slot
  - causal masking via gpsimd affine_select (frees DVE, no tril table)
  - reciprocal broadcast via gpsimd partition_broadcast (no PSUM traffic)
  - biases folded into the projection as K=1 rank-1 matmuls; psum drains
    become plain DVE casts
  - V transposes + previous-tile output projection live in the block-tail
    "reciprocal window" where score psum slots are free
  - HAM warmup with real matmuls (transposes don't engage the clock gate)
"""

import numpy as np
import ml_dtypes
from contextlib import ExitStack

import concourse.bass as bass
from concourse import bacc
import concourse.mybir as mybir
import concourse.tile as tile
from concourse.tile import add_dep_helper
from concourse.bass_utils import run_bass_kernel_spmd

f32 = mybir.dt.float32
bf16 = mybir.dt.bfloat16
MDT = bf16
NPBF = ml_dtypes.bfloat16

S = 2048
D = 1024
HEADS = 16
HD = 64
KVH = 4
N_CORES = 8

ST = 512          # i-tile (free dim of most matmuls)
NS = S // ST      # 4
FP = 128          # contraction chunk
NF = D // FP      # 8
JTS = 128         # j-chunk (key positions per score tile partition dim)
NJ = S // JTS     # 16
NE = D // 128     # 8 output-feature chunks

CHAIN_DEPTH = 3   # in-flight input DMAs (arrival-order enforcement)

_CACHE = {}


def _build_program():
    key = "nc"
    if key in _CACHE:
        return _CACHE[key]

    nc = bacc.Bacc("TRN2", target_bir_lowering=False, debug=False)

    def din(name, shape, dt=MDT):
        return nc.dram_tensor(name, shape, dt, kind="ExternalInput").ap()

    # inputs pre-chunked on host: [s-tile][partition][f * 512] contiguous
    qT = din("qT", [NS, 128, NF * ST])
    kT = din("kT", [NS, 128, NF * ST])
    vT = din("vT", [NS, 128, NF * ST])
    wq = din("wq", [128, NF * 128])
    wk = din("wk", [128, NF * 64])
    wv = din("wv", [128, NF * 64])
    wo = din("wo", [128, D])
    bqr = din("bqr", [1, 128])            # bias rows (K=1 matmul operands)
    bkvr = din("bkvr", [1, 128])          # cols 0:64 = bv, 64:128 = bk
    cosk = din("cosk", [32, S])
    sink = din("sink", [32, S])
    ident_in = din("ident", [64, 64])
    outT = nc.dram_tensor("outT", [D, S], MDT, kind="ExternalOutput").ap()

    Exp = mybir.ActivationFunctionType.Exp
    GE = mybir.AluOpType.is_ge

    with tile.TileContext(nc) as tc, ExitStack() as ctx, \
            nc.allow_low_precision(reason="bf16 kernel by design"):
        const = ctx.enter_context(tc.tile_pool(name="const", bufs=1))
        big = ctx.enter_context(tc.tile_pool(name="big", bufs=1))
        stream = ctx.enter_context(tc.tile_pool(name="stream", bufs=1))
        ptile = ctx.enter_context(tc.tile_pool(name="ptile", bufs=1))
        small = ctx.enter_context(tc.tile_pool(name="small", bufs=1))
        outb = ctx.enter_context(tc.tile_pool(name="outb", bufs=1))
        psum = ctx.enter_context(tc.tile_pool(name="psum", bufs=1, space="PSUM"))

        def mm(out, lhsT, rhs, start, stop, tp=None, skip=False):
            nc.tensor.matmul(out, lhsT=lhsT, rhs=rhs, start=start, stop=stop,
                             tile_position=tp, skip_group_check=skip)

        # ---- tiny constants, unchained (land in ~1us) ----
        ident = const.tile([64, 64], MDT)
        nc.sync.dma_start(out=ident, in_=ident_in)
        wv_sb = const.tile([128, NF, 64], MDT)
        nc.sync.dma_start(out=wv_sb, in_=wv.rearrange("p (f d) -> p f d", f=NF))
        wk_sb = const.tile([128, NF, 64], MDT)
        nc.sync.dma_start(out=wk_sb, in_=wk.rearrange("p (f d) -> p f d", f=NF))
        bqr_sb = const.tile([1, 128], MDT)
        nc.sync.dma_start(out=bqr_sb, in_=bqr)
        bkvr_sb = const.tile([1, 128], MDT)
        nc.sync.dma_start(out=bkvr_sb, in_=bkvr)

        cos_sb = const.tile([128, S], MDT)
        sin_sb = const.tile([128, S], MDT)
        wq_sb = const.tile([128, NF, 128], MDT)
        wo_sb = const.tile([128, D], MDT)

        qh = big.tile([128, S], MDT)
        khT2 = big.tile([128, S], MDT)
        vhT = big.tile([64, S], MDT)
        vh_aug = big.tile([128, NJ, 65], MDT)
        attn = big.tile([128, S], MDT)
        nc.vector.memset(vh_aug[:, :, 64], 1.0)
        ones512 = const.tile([1, ST], MDT)
        nc.vector.memset(ones512, 1.0)
        ones16 = const.tile([1, 64], MDT)
        nc.vector.memset(ones16, 1.0)
        warm_src = const.tile([64, 512], MDT)
        nc.vector.memset(warm_src, 0.0)

        # ---- HAM warmup: real matmuls on zero data spanning the initial
        # input-DMA wait, so the first projection matmuls run at 2.4 GHz ----
        wp = psum.tile([128, 2, ST], f32, tag="mm", bufs=2, name="warm")
        for _ in range(12):
            mm(wp[0:64, 0, :], warm_src[:, 0:64], warm_src,
               start=True, stop=True, tp=(0, 0))

        # ---- input DMAs: group-chained per s-tile.  Group s waits on the
        # first DMA of group s-1, so s-tiles arrive in need order at full
        # aggregate bandwidth (free-running round-robin would invert it) ----
        xs = {}
        anchors = []

        def in_group(s, extra=()):
            xs[s] = (stream.tile([128, NF, ST], MDT, tag="xv", bufs=2,
                                 name="xv"),
                     stream.tile([128, NF, ST], MDT, tag="xk", bufs=2,
                                 name="xk"),
                     stream.tile([128, NF, ST], MDT, tag="xq", bufs=2,
                                 name="xq"))
            xv, xk, xq = xs[s]
            insts = []
            for h in range(2):
                fs = slice(4 * h, 4 * h + 4)
                insts.append(nc.sync.dma_start(
                    out=xv[:, fs, :],
                    in_=vT[s].rearrange("p (f c) -> p f c", f=NF)[:, fs, :]).ins)
                insts.append(nc.sync.dma_start(
                    out=xk[:, fs, :],
                    in_=kT[s].rearrange("p (f c) -> p f c", f=NF)[:, fs, :]).ins)
                insts.append(nc.gpsimd.dma_start(
                    out=xq[:, fs, :],
                    in_=qT[s].rearrange("p (f c) -> p f c", f=NF)[:, fs, :]).ins)
            for eng, out, in_ in extra:
                insts.append(eng.dma_start(out=out, in_=in_).ins)
            if anchors:
                for i in insts:
                    add_dep_helper(i, anchors[-1], reason="s-tile order")
            anchors.append(insts[0])

        # tables ride with s0 (needed first); wo with s1 (needed ~3rd block)
        in_group(0, extra=[
            (nc.scalar, wq_sb, wq.rearrange("p (f d) -> p f d", f=NF)),
            (nc.scalar, cos_sb[0:32, :], cosk),
            (nc.scalar, sin_sb[32:64, :], sink)])
        in_group(1)
        in_group(2, extra=[(nc.scalar, wo_sb, wo)])
        in_group(3)

        # expand the 32-row tables to the 4-block layout on the idle early
        # DVE: cos -> [c,c,c,c], sin -> [-s,s,-s,s]
        for dst in (32, 64, 96):
            nc.vector.tensor_copy(cos_sb[dst:dst + 32, :], cos_sb[0:32, :])
        nc.vector.tensor_scalar_mul(sin_sb[0:32, :], sin_sb[32:64, :], -1.0)
        nc.vector.tensor_copy(sin_sb[64:96, :], sin_sb[0:32, :])
        nc.vector.tensor_copy(sin_sb[96:128, :], sin_sb[32:64, :])

        def proj_units(s, st):
            # Q first: the next block's sub-diagonal scores only need roped
            # q (+ old k tiles), so fin_q lands mid-feed and fin_kv (K-rope,
            # V transposes) can finish during the next block's early slots
            xv, xk, xq = xs[s]
            units = []
            for f in range(NF):
                def uq(f=f):
                    if "q" not in st:
                        st["q"] = psum.tile([128, ST], f32, tag="acc",
                                            bufs=4, name="psq")
                    mm(st["q"], wq_sb[:, f, :], xq[:, f, :],
                       start=(f == 0), stop=False)
                units.append(uq)

            def uqb():
                mm(st["q"], bqr_sb, ones512, start=False, stop=True, skip=True)
            units.append(uqb)

            def finq():
                fin_q_a(s, st)
                fin_q_b(s, st)
            units.append(finq)
            for f in range(NF):
                def ukv(f=f):
                    if "kv" not in st:
                        st["kv"] = psum.tile([128, ST], f32, tag="acc",
                                             bufs=4, name="pskv")
                    mm(st["kv"][0:64, :], wv_sb[:, f, :], xv[:, f, :],
                       start=(f == 0), stop=False, tp=(0, 0))
                    mm(st["kv"][64:128, :], wk_sb[:, f, :], xk[:, f, :],
                       start=(f == 0), stop=False, tp=(0, 64), skip=True)
                units.append(ukv)

            def ukvb():
                mm(st["kv"], bkvr_sb, ones512, start=False, stop=True,
                   skip=True)
            units.append(ukvb)

            def finkv():
                fin_kv_a(s, st)
                fin_kv_b(s, st)
            units.append(finkv)
            return units

        def fin_kv_a(s, st):
            # kv psum drains + V transposes
            ssl = slice(s * ST, (s + 1) * ST)
            nc.vector.tensor_copy(vhT[:, ssl], st["kv"][0:64, :])
            nc.vector.tensor_copy(khT2[64:128, ssl], st["kv"][64:128, :])
            tp_units(s)

        def fin_kv_b(s, st):
            # K-RoPE + GQA dup
            ssl = slice(s * ST, (s + 1) * ST)
            ksw = ptile.tile([128, ST], MDT, tag="ksw", bufs=2)
            nc.vector.tensor_copy(ksw[64:96, :], khT2[96:128, ssl])
            nc.vector.tensor_copy(ksw[96:128, :], khT2[64:96, ssl])
            nc.vector.tensor_mul(ksw[64:128, :], ksw[64:128, :],
                                 sin_sb[64:128, ssl])
            nc.vector.tensor_mul(khT2[64:128, ssl], khT2[64:128, ssl],
                                 cos_sb[64:128, ssl])
            nc.vector.tensor_add(khT2[64:128, ssl], khT2[64:128, ssl],
                                 ksw[64:128, :])
            # duplicate roped kv head into partitions 0:64 for head-0 scores
            nc.vector.tensor_copy(khT2[0:64, ssl], khT2[64:128, ssl])

        def fin_q_a(s, st):
            ssl = slice(s * ST, (s + 1) * ST)
            nc.vector.tensor_copy(qh[:, ssl], st["q"])
            qsw = ptile.tile([128, ST], MDT, tag="qsw", bufs=2)
            for (dstp, srcp) in ((0, 32), (32, 0), (64, 96), (96, 64)):
                nc.vector.tensor_copy(qsw[dstp:dstp + 32, :],
                                      qh[srcp:srcp + 32, ssl])
            st["qsw"] = qsw

        def fin_q_b(s, st):
            ssl = slice(s * ST, (s + 1) * ST)
            qsw = st["qsw"]
            nc.vector.tensor_mul(qsw, qsw, sin_sb[:, ssl])
            nc.vector.tensor_mul(qh[:, ssl], qh[:, ssl], cos_sb[:, ssl])
            nc.vector.tensor_add(qh[:, ssl], qh[:, ssl], qsw)

        def tp_units(s):
            # V transpose into vh_aug; runs in the block-tail window on the
            # acc psum slots freed by the mid-block projection drains, so
            # the score (mm) slots stay free for the next block
            for m in range(4):
                jt = 4 * s + m
                tp_ps = psum.tile([128, 64], MDT, tag="acc", bufs=4,
                                  name="tp")
                nc.tensor.transpose(tp_ps,
                                    vhT[:, jt * JTS:(jt + 1) * JTS], ident)
                nc.vector.tensor_copy(vh_aug[:, jt, 0:64], tp_ps)

        ob_chain = []

        def oproj_units(it, last=False):
            isl = slice(it * ST, (it + 1) * ST)
            units = []
            obs = {}

            def u(e):
                pw = psum.tile([128, ST], f32, tag="acc", bufs=4, name="pw")
                mm(pw, wo_sb[:, e * 128:(e + 1) * 128], attn[:, isl],
                   start=True, stop=True)
                if e % 2 == 0:
                    obs["ob"] = outb.tile([128, 2, ST], MDT, tag="ob", bufs=3,
                                          name="ob")
                ob = obs["ob"]
                act_drain = (e % 2 == 0) if last else (e % 4 == 0)
                if act_drain:
                    nc.scalar.copy(ob[:, e % 2, :], pw)
                else:
                    nc.vector.tensor_copy(ob[:, e % 2, :], pw)
                if e % 2 == 1:
                    ep = e // 2
                    odst = outT.rearrange("(g p) s -> p g s", p=128) \
                        [:, 2 * ep:2 * ep + 2, isl]
                    if last and ep % 2 == 0:
                        nc.sync.dma_start(out=odst, in_=ob)
                    else:
                        inst = nc.gpsimd.dma_start(out=odst, in_=ob).ins
                        if len(ob_chain) >= 2:
                            add_dep_helper(inst, ob_chain[-2],
                                           reason="limit output inflight")
                        ob_chain.append(inst)
            for e in range(NE):
                units.append(lambda e=e: u(e))
            return units

        def attn_block(it, feed=(), tailwork=()):
            # scores for jt+1 issue before PV of jt so the PE streams through
            # the exp latency; `feed` units run woven between slots
            feed = list(feed)
            po0 = psum.tile([65, ST], f32, tag="acc", bufs=4, name="po0")
            po1 = psum.tile([65, ST], f32, tag="acc", bufs=4, name="po1")
            jmax = 4 * it + 3
            pts = {}

            def sc(jt):
                lo = (jt - 4 * it) * JTS if jt >= 4 * it else 0
                jsl = slice(jt * JTS, (jt + 1) * JTS)
                isl = slice(it * ST + lo, (it + 1) * ST)
                pair = psum.tile([128, 2, ST], f32, tag="mm", bufs=2,
                                 name="pair")
                mm(pair[:, 0, lo:], khT2[0:64, jsl], qh[0:64, isl],
                   start=True, stop=True, tp=(0, 0))
                mm(pair[:, 1, lo:], khT2[64:128, jsl], qh[64:128, isl],
                   start=True, stop=True, tp=(64, 0))
                pt = ptile.tile([128, 2, ST], MDT, tag="pt", bufs=4)
                nc.scalar.activation(out=pt[:, :, lo:], in_=pair[:, :, lo:],
                                     func=Exp, scale=0.125)
                if jt >= 4 * it:
                    # causal mask on the diagonal 128-block, both heads
                    nc.gpsimd.affine_select(
                        out=pt[:, :, lo:lo + JTS], in_=pt[:, :, lo:lo + JTS],
                        pattern=[[0, 2], [1, JTS]], compare_op=GE, fill=0.0,
                        base=0, channel_multiplier=-1)
                pts[jt] = (pt, lo)

            def pv(jt):
                pt, lo = pts.pop(jt)
                mm(po0[:, lo:], vh_aug[:, jt, :], pt[:, 0, lo:],
                   start=(jt == 0), stop=(jt == jmax))
                mm(po1[:, lo:], vh_aug[:, jt, :], pt[:, 1, lo:],
                   start=(jt == 0), stop=(jt == jmax))

            sc(0)
            for jt in range(jmax + 1):
                if jt < jmax:
                    sc(jt + 1)
                if feed and jt >= 1:
                    n = -(-len(feed) // (jmax + 1 - jt))
                    for u in feed[:n]:
                        u()
                    feed = feed[n:]
                pv(jt)
            for u in feed:
                u()

            # ---- tail window: denominators -> reciprocal -> broadcast,
            # with V transposes + previous oproj filling the PE ----
            isl = slice(it * ST, (it + 1) * ST)
            sums = small.tile([1, 2, ST], f32, tag="sums", bufs=2)
            rc = small.tile([1, 2, ST], f32, tag="rc", bufs=2)
            rcb16 = small.tile([1, 2, ST], MDT, tag="rcb16", bufs=2)
            nc.scalar.copy(sums[:, 0, :], po0[64:65, :])
            nc.scalar.copy(sums[:, 1, :], po1[64:65, :])
            nc.vector.reciprocal_approx_fast(rc, sums)
            nc.vector.tensor_copy(rcb16, rc)
            bct = ptile.tile([128, ST], MDT, tag="bct", bufs=2)
            bp = psum.tile([128, ST], f32, tag="acc", bufs=4, name="bp")
            mm(bp[0:64, :], ones16, rcb16[:, 0, :],
               start=True, stop=True, tp=(0, 0))
            mm(bp[64:128, :], ones16, rcb16[:, 1, :],
               start=True, stop=True, tp=(0, 64), skip=True)
            nc.scalar.copy(bct, bp)
            for u in tailwork:
                u()
            nc.vector.tensor_mul(attn[0:64, isl], po0[0:64, :], bct[0:64, :])
            nc.vector.tensor_mul(attn[64:128, isl], po1[0:64, :],
                                 bct[64:128, :])

        # ---- pipeline ----
        st0 = {}
        for u in proj_units(0, st0):
            u()
        sts = {0: st0}
        for s in range(1, NS):
            sts[s] = {}
            feed = proj_units(s, sts[s])
            tail = oproj_units(s - 2) if s >= 2 else []
            attn_block(s - 1, feed=feed, tailwork=tail)
        attn_block(NS - 1, feed=oproj_units(NS - 2))
        for u in oproj_units(NS - 1, last=True):
            u()

    nc.compile()
    _CACHE[key] = nc
    return nc


def _host_tables():
    if "tables" in _CACHE:
        return _CACHE["tables"]
    # faithful to reference: exp = -2*arange(0,64,2)/64
    expv = -2.0 * np.arange(0, HD, 2, dtype=np.float32) / HD
    thetas = np.power(np.float32(10000.0), expv).astype(np.float32)    # [32]
    m = np.arange(S, dtype=np.float32)
    freq = np.outer(m, thetas).astype(np.float32)                      # [S, 32]
    cos = np.cos(freq).astype(np.float32).T                            # [32, S]
    sin = np.sin(freq).astype(np.float32).T
    perm = np.concatenate([np.arange(0, HD, 2), np.arange(1, HD, 2)])  # deint
    _CACHE["tables"] = (
        np.ascontiguousarray(cos.astype(NPBF)),
        np.ascontiguousarray(sin.astype(NPBF)),
        perm,
    )
    return _CACHE["tables"]


def _warr(w):
    # [1024, nd] -> [128, NF*nd] with chunk-of-128-rows as middle dim
    nd = w.shape[1]
    return np.ascontiguousarray(
        w.reshape(NF, FP, nd).transpose(1, 0, 2).reshape(FP, NF * nd)
        .astype(NPBF))


def kernel(**inputs):
    q = np.asarray(inputs["q"], np.float32)[0]       # [S, D]
    k = np.asarray(inputs["k"], np.float32)[0]
    v = np.asarray(inputs["v"], np.float32)[0]
    Wq = np.asarray(inputs["Wq"], np.float32)
    Wk = np.asarray(inputs["Wk"], np.float32)
    Wv = np.asarray(inputs["Wv"], np.float32)
    Wo = np.asarray(inputs["Wo"], np.float32)
    bq = np.asarray(inputs["bq"], np.float32)
    bk = np.asarray(inputs["bk"], np.float32)
    bv = np.asarray(inputs["bv"], np.float32)
    bo = np.asarray(inputs["bo"], np.float32)

    cos32, sin32, perm = _host_tables()

    # head_dim deinterleave permutation applied to q/k projection columns
    permQ = np.concatenate([h * HD + perm for h in range(HEADS)])
    permK = np.concatenate([g * HD + perm for g in range(KVH)])
    Wqp = Wq[:, permQ]
    bqp = bq[permQ]
    Wkp = Wk[:, permK]
    bkp = bk[permK]

    def chunk(x):
        # [S, D] -> [NS, 128, NF*512]: xc[s, p, f*512+c] = x[s*512+c, f*128+p]
        xc = x.T.reshape(NF, FP, NS, ST).transpose(2, 1, 0, 3)
        return np.ascontiguousarray(
            xc.reshape(NS, FP, NF * ST).astype(NPBF))

    qT = chunk(q)
    kT = chunk(k)
    vT = chunk(v)
    ident64 = np.eye(64, dtype=np.float32).astype(NPBF)

    in_maps = []
    for c in range(N_CORES):
        g = c // 2
        if c % 2 == 0:
            h0, h1 = g, g + 4
        else:
            h0, h1 = g + 8, g + 12
        wq_c = np.concatenate([Wqp[:, h0 * HD:(h0 + 1) * HD],
                               Wqp[:, h1 * HD:(h1 + 1) * HD]], axis=1)
        bq_c = np.ascontiguousarray(
            np.concatenate([bqp[h0 * HD:(h0 + 1) * HD],
                            bqp[h1 * HD:(h1 + 1) * HD]]).reshape(1, 128)
            .astype(NPBF))
        bkv_c = np.ascontiguousarray(
            np.concatenate([bv[g * HD:(g + 1) * HD],
                            bkp[g * HD:(g + 1) * HD]]).reshape(1, 128)
            .astype(NPBF))
        wo_c = np.ascontiguousarray(
            np.concatenate([Wo[h0 * HD:(h0 + 1) * HD, :],
                            Wo[h1 * HD:(h1 + 1) * HD, :]], axis=0)
            .astype(NPBF))

        in_maps.append({
            "qT": qT, "kT": kT, "vT": vT,
            "wq": _warr(wq_c),
            "wk": _warr(Wkp[:, g * HD:(g + 1) * HD]),
            "wv": _warr(Wv[:, g * HD:(g + 1) * HD]),
            "wo": wo_c,
            "bqr": bq_c,
            "bkvr": bkv_c,
            "cosk": cos32, "sink": sin32,
            "ident": ident64,
        })

    nc = _build_program()
    res = run_bass_kernel_spmd(nc, in_maps, list(range(N_CORES)))
    acc = np.zeros((D, S), np.float32)
    for r in res.results:
        acc += np.asarray(r["outT"], np.float32)
    out = acc.T + bo[None, :]
    return out[None].astype(np.float32)
